# revision 8
# baseline (speedup 1.0000x reference)
"""GQA attention with KV cache, tensor-parallel over 8 TRN2 NeuronCores.

Problem shapes (hardcoded): H=32 q-heads, KVH=8 kv-heads, D=128 head_dim,
DIM=4096, T=256 new tokens, MAX_SEQ=8192, pos=4096 (runtime input).

Sharding: head-parallel. Core c owns q-heads 4c..4c+3 and kv-head c:
  wq rows  [c*512:(c+1)*512], wk/wv rows [c*128:(c+1)*128],
  wo cols  [c*512:(c+1)*512], k/v_cache head c.
Each core computes a full (T, DIM) partial of the output projection;
the host sums the 8 partials (the TP all-reduce) and reshapes.

Weights are pre-transposed on the host (K-major for the PE contraction
dim) — standard offline weight prep. x and the KV cache are loaded in
their natural layouts and transposed on-chip with PE transposes.

Per-core dataflow (all matmuls in float32r, 1 cyc/row at N>=256):
  x (T,DIM) --PE transpose--> xT (DIM,T)
  qT_h = wqT_h.T @ xT ; kT/vT likewise      (PSUM accumulation over DIM)
  RoPE on qT/kT via R-rotation matmul + DVE combine (cos/sin tables
  host-transposed; q tables pre-scaled by 1/sqrt(D))
  scoresT[s-block] = kT[:,s-block].T @ qrT_all   (4 heads batched, N=1024)
  probsT = exp(scoresT)            (ACT, PSUM->SBUF, no max subtraction:
                                    |scores*scale| <~ 8 so exp is safe in fp32)
  row-sums: DVE accumulation of probsT blocks, then ones-matmul partition
  reduce; reciprocal; gpsimd partition_broadcast
  outT_h += v[s-block].T @ probsT  (v natural from cache, PSUM accum)
  attnoutT = outT * rinv_bcast     (softmax normalization, post-PV)
  out_partial = attnoutT.T @ woT   (accumulate 4 head-blocks in PSUM)
"""

import numpy as np

import concourse.mybir as mybir
import concourse.tile as tile
from concourse import bacc
from concourse.bass_utils import run_bass_kernel_spmd
from concourse.masks import make_identity

H, KVH, D = 32, 8, 128
DIM, T, MAX_SEQ = 4096, 256, 8192
NC_ = 8                      # cores
HL = H // NC_                # local q heads = 4
SCALE = 1.0 / float(np.sqrt(D))

F32 = mybir.dt.float32
F32R = mybir.dt.float32r
MM_DT = F32R                 # matmul operand dtype (float32r: full-rate fp32)

_BUILD_CACHE: dict = {}


def _build(pos: int):
    """Trace + compile the per-core program. Same program runs on all 8
    cores (SPMD); only the DRAM input contents differ."""
    S_OLD = pos              # cached tokens
    S = pos + T              # total keys
    NB_OLD = S_OLD // 128    # cached s-blocks (32)
    NB = S // 128            # total s-blocks (34)
    NKT = DIM // 128         # contraction k-tiles (32)
    NT = T // 128            # t-tiles (2)

    nc = bacc.Bacc("TRN2", target_bir_lowering=False, debug=False)

    # DRAM I/O (per-core shards; host prepares layouts)
    d_x = nc.dram_tensor("x", (T, DIM), MM_DT, kind="ExternalInput")
    d_wq = nc.dram_tensor("wqT", (DIM, HL * D), MM_DT, kind="ExternalInput")
    d_wk = nc.dram_tensor("wkT", (DIM, D), MM_DT, kind="ExternalInput")
    d_wv = nc.dram_tensor("wvT", (DIM, D), MM_DT, kind="ExternalInput")
    d_wo = nc.dram_tensor("woT", (HL * D, DIM), MM_DT, kind="ExternalInput")
    d_kc = nc.dram_tensor("kc", (S_OLD, D), MM_DT, kind="ExternalInput")
    d_vc = nc.dram_tensor("vc", (S_OLD, D), MM_DT, kind="ExternalInput")
    d_cq = nc.dram_tensor("cosq", (D, T), MM_DT, kind="ExternalInput")
    d_sq = nc.dram_tensor("sinq", (D, T), MM_DT, kind="ExternalInput")
    d_ck = nc.dram_tensor("cosk", (D, T), MM_DT, kind="ExternalInput")
    d_sk = nc.dram_tensor("sink", (D, T), MM_DT, kind="ExternalInput")
    d_out = nc.dram_tensor("out", (T, DIM), F32, kind="ExternalOutput")

    with tile.TileContext(nc) as tc:
        with (
            tc.tile_pool(name="persist", bufs=1) as pp,
            tc.tile_pool(name="wstream", bufs=3) as wp,   # 16KB/slot stream
            tc.tile_pool(name="small", bufs=2) as sp,
            tc.tile_pool(name="probs", bufs=3) as prp,
        ):
            # ---- constants ----
            # gpsimd memset/affine_select emit f32; round into f32r tiles
            # via DVE copies (satisfies the fp32r-producer-rounding rule).
            ident = pp.tile([128, 128], MM_DT, tag="ident")
            scr_i = sp.tile([128, 128], F32, tag="cscr", name="scr_ident")
            make_identity(nc, scr_i[:])
            nc.vector.tensor_copy(ident[:], scr_i[:])
            # R^T for rotate_half: +1 at (x, x+64), -1 at (x, x-64)
            rt = pp.tile([128, 128], MM_DT, tag="rt")
            scr_r = sp.tile([128, 128], F32, tag="cscr", name="scr_rt")
            nc.gpsimd.memset(scr_r[:], 0.0)
            nc.gpsimd.affine_select(
                out=scr_r[:], in_=scr_r[:], compare_op=mybir.AluOpType.not_equal,
                fill=1.0, base=64, channel_multiplier=1, pattern=[[-1, 128]],
            )
            nc.gpsimd.affine_select(
                out=scr_r[:], in_=scr_r[:], compare_op=mybir.AluOpType.not_equal,
                fill=-1.0, base=-64, channel_multiplier=1, pattern=[[-1, 128]],
            )
            nc.vector.tensor_copy(rt[:], scr_r[:])
            ones_col = pp.tile([128, 1], MM_DT, tag="ones")
            scr_o = sp.tile([128, 1], F32, tag="cscr1", name="scr_ones")
            nc.gpsimd.memset(scr_o[:], 1.0)
            nc.vector.tensor_copy(ones_col[:], scr_o[:])

            # cos/sin tables (host-transposed; q tables pre-scaled)
            cs = {}
            for nm, dt_ in (("cosq", d_cq), ("sinq", d_sq),
                            ("cosk", d_ck), ("sink", d_sk)):
                t_ = pp.tile([D, T], MM_DT, tag=nm, name=f"cs_{nm}")
                nc.sync.dma_start(t_[:], dt_.ap()[:, :])
                cs[nm] = t_

            # persistent activations
            v_all = pp.tile([128, NB * D], MM_DT, tag="vall")
            nc.sync.dma_start(
                v_all[:, 0:NB_OLD * D].rearrange("p (n d) -> p n d", d=D),
                d_vc.ap().rearrange("(n p) d -> p n d", p=128))
            xT = pp.tile([128, NKT * T], MM_DT, tag="xT")         # [dim, k*T+t]
            kT_all = pp.tile([128, S], MM_DT, tag="kT")           # [d, s]
            qrT = pp.tile([128, HL * T], MM_DT, tag="qrT")        # [d, h*T+t]
            acc_sum = pp.tile([128, HL * T], MM_DT, tag="accsum")
            attnT = pp.tile([128, HL * T], MM_DT, tag="attnT")
            rinv = pp.tile([1, HL * T], MM_DT, tag="rinv")
            rinv_bc = pp.tile([128, HL * T], MM_DT, tag="rinvbc")

            # ================= phase A: transposes + QKV + RoPE ============
            with (
                tc.tile_pool(name="ps_tr", bufs=2, space="PSUM") as ps_tr,
                tc.tile_pool(name="ps_acc", bufs=3, space="PSUM") as ps_acc,
            ):
                # x -> xT (x streamed through the 16KB "w" slots)
                for i in range(NT):
                    x_nat = wp.tile([128, DIM], MM_DT, tag="w", name=f"xn{i}")
                    nc.sync.dma_start(
                        x_nat[:], d_x.ap()[i * 128:(i + 1) * 128, :])
                    for k in range(NKT):
                        p = ps_tr.tile([128, 128], MM_DT, tag="tr")
                        nc.tensor.transpose(
                            p[:], x_nat[:, k * 128:(k + 1) * 128], ident[:])
                        nc.vector.tensor_copy(
                            xT[:, k * T + i * 128: k * T + (i + 1) * 128], p[:])

                def project(w_sb):
                    """Accumulate acc = w_sb.T @ xT over k-tiles -> psum."""
                    acc = ps_acc.tile([128, T], F32, tag="acc")
                    for k in range(NKT):
                        nc.tensor.matmul(
                            acc[:], w_sb[:, k, :], xT[:, k * T:(k + 1) * T],
                            start=(k == 0), stop=(k == NKT - 1),
                        )
                    return acc

                def rope(acc_ps, cos_t, sin_t, dest_ap):
                    """dest = cos*z + sin*(R@z), z = acc_ps ([d, T])."""
                    z_sb = sp.tile([128, T], MM_DT, tag="z")
                    nc.vector.tensor_copy(z_sb[:], acc_ps[:])
                    rz = ps_acc.tile([128, T], F32, tag="acc")
                    nc.tensor.matmul(rz[:], rt[:], z_sb[:], start=True, stop=True)
                    m1 = sp.tile([128, T], MM_DT, tag="m1")
                    nc.vector.tensor_mul(m1[:], z_sb[:], cos_t[:])
                    m2 = sp.tile([128, T], MM_DT, tag="m2")
                    nc.vector.tensor_mul(m2[:], rz[:], sin_t[:])
                    nc.vector.tensor_add(dest_ap, m1[:], m2[:])

                # q heads
                for h in range(HL):
                    w_sb = wp.tile([128, NKT, 128], MM_DT, tag="w",
                                   name=f"wq{h}")
                    nc.sync.dma_start(
                        w_sb[:],
                        d_wq.ap()[:, h * 128:(h + 1) * 128]
                        .rearrange("(k p) m -> p k m", p=128))
                    acc = project(w_sb)
                    rope(acc, cs["cosq"], cs["sinq"],
                         qrT[:, h * T:(h + 1) * T])
                # k
                w_sb = wp.tile([128, NKT, 128], MM_DT, tag="w", name="wk")
                nc.sync.dma_start(
                    w_sb[:], d_wk.ap().rearrange("(k p) m -> p k m", p=128))
                acc = project(w_sb)
                rope(acc, cs["cosk"], cs["sink"], kT_all[:, S_OLD:S])
                # v (vT -> PE transpose -> natural v blocks)
                w_sb = wp.tile([128, NKT, 128], MM_DT, tag="w", name="wv")
                nc.sync.dma_start(
                    w_sb[:], d_wv.ap().rearrange("(k p) m -> p k m", p=128))
                acc = project(w_sb)
                vT_sb = sp.tile([128, T], MM_DT, tag="z")
                nc.vector.tensor_copy(vT_sb[:], acc[:])
                for i in range(NT):
                    p = ps_tr.tile([128, 128], MM_DT, tag="tr")
                    nc.tensor.transpose(
                        p[:], vT_sb[:, i * 128:(i + 1) * 128], ident[:])
                    nc.vector.tensor_copy(
                        v_all[:, (NB_OLD + i) * D:(NB_OLD + i + 1) * D], p[:])
                # k_cache -> kT_all[:, :S_OLD]
                kc_nat = wp.tile([128, NB_OLD, D], MM_DT, tag="w", name="kcn")
                nc.sync.dma_start(
                    kc_nat[:], d_kc.ap().rearrange("(n p) d -> p n d", p=128))
                for n in range(NB_OLD):
                    p = ps_tr.tile([128, 128], MM_DT, tag="tr")
                    nc.tensor.transpose(p[:], kc_nat[:, n, :], ident[:])
                    nc.vector.tensor_copy(
                        kT_all[:, n * 128:(n + 1) * 128], p[:])

            # ================= phase B: attention =========================
            with tc.tile_pool(name="ps_pv", bufs=1, space="PSUM") as ps_pv:
                pv = ps_pv.tile([128, HL * T], F32, tag="pv")
                with tc.tile_pool(name="ps_sc", bufs=2, space="PSUM") as ps_sc:
                    for s in range(NB):
                        sc = ps_sc.tile([128, HL * T], F32, tag="sc")
                        for half in range(2):
                            nc.tensor.matmul(
                                sc[:, half * 512:(half + 1) * 512],
                                kT_all[:, s * 128:(s + 1) * 128],
                                qrT[:, half * 512:(half + 1) * 512],
                                start=True, stop=True)
                        pb = prp.tile([128, HL * T], MM_DT, tag="pb")
                        nc.scalar.activation(
                            pb[:], sc[:], mybir.ActivationFunctionType.Exp)
                        if s == 0:
                            nc.vector.tensor_copy(acc_sum[:], pb[:])
                        else:
                            nc.vector.tensor_add(acc_sum[:], acc_sum[:], pb[:])
                        for half in range(2):
                            nc.tensor.matmul(
                                pv[:, half * 512:(half + 1) * 512],
                                v_all[:, s * D:(s + 1) * D],
                                pb[:, half * 512:(half + 1) * 512],
                                start=(s == 0), stop=(s == NB - 1))

                # softmax denominators + normalization
                with tc.tile_pool(name="ps_post", bufs=2, space="PSUM") as psp:
                    for half in range(2):
                        sm = psp.tile([1, 512], F32, tag="sm")
                        nc.tensor.matmul(
                            sm[:], ones_col[:],
                            acc_sum[:, half * 512:(half + 1) * 512],
                            start=True, stop=True)
                        with nc.allow_low_precision(
                                reason="softmax denom reciprocal in f32r"):
                            nc.vector.reciprocal(
                                rinv[:, half * 512:(half + 1) * 512], sm[:])
                    nc.gpsimd.partition_broadcast(rinv_bc[:], rinv[:])
                    for half in range(2):
                        nc.vector.tensor_mul(
                            attnT[:, half * 512:(half + 1) * 512],
                            pv[:, half * 512:(half + 1) * 512],
                            rinv_bc[:, half * 512:(half + 1) * 512])

            # ================= phase C: output projection =================
            # wo streamed in (h, n)-tiled [128, 512] pieces; per output
            # n-chunk, accumulate the 4 head-blocks in PSUM.
            with (
                tc.tile_pool(name="ps_wo", bufs=2, space="PSUM") as ps_wo,
                tc.tile_pool(name="wotile", bufs=8) as wop,
            ):
                for n in range(DIM // 512):
                    wo_t = []
                    for h in range(HL):
                        w_sb = wop.tile([128, 512], MM_DT, tag="wot",
                                        name=f"wo{h}_{n}")
                        nc.sync.dma_start(
                            w_sb[:],
                            d_wo.ap()[h * 128:(h + 1) * 128,
                                      n * 512:(n + 1) * 512])
                        wo_t.append(w_sb)
                    for i in range(NT):
                        po = ps_wo.tile([128, 512], F32, tag="po")
                        for h in range(HL):
                            nc.tensor.matmul(
                                po[:],
                                attnT[:, h * T + i * 128: h * T + (i + 1) * 128],
                                wo_t[h][:],
                                start=(h == 0), stop=(h == HL - 1))
                        ob = sp.tile([128, 512], F32, tag="ob")
                        nc.vector.tensor_copy(ob[:], po[:])
                        nc.sync.dma_start(
                            d_out.ap()[i * 128:(i + 1) * 128,
                                       n * 512:(n + 1) * 512], ob[:])

    nc.compile()
    return nc


def _prep_inputs(x, cos, sin, wq, wk, wv, wo, k_cache, v_cache, pos):
    """Host-side shard + layout prep. Returns in_maps for the 8 cores."""
    f = np.float32
    x2d = np.ascontiguousarray(np.asarray(x).reshape(T, DIM), dtype=f)
    cos = np.asarray(cos)
    sin = np.asarray(sin)
    cosq = np.ascontiguousarray(cos.T * SCALE, dtype=f)
    sinq = np.ascontiguousarray(sin.T * SCALE, dtype=f)
    cosk = np.ascontiguousarray(cos.T, dtype=f)
    sink = np.ascontiguousarray(sin.T, dtype=f)
    pos = int(pos)
    in_maps = []
    for c in range(NC_):
        in_maps.append({
            "x": x2d,
            "wqT": np.ascontiguousarray(
                np.asarray(wq)[c * HL * D:(c + 1) * HL * D, :].T, dtype=f),
            "wkT": np.ascontiguousarray(
                np.asarray(wk)[c * D:(c + 1) * D, :].T, dtype=f),
            "wvT": np.ascontiguousarray(
                np.asarray(wv)[c * D:(c + 1) * D, :].T, dtype=f),
            "woT": np.ascontiguousarray(
                np.asarray(wo)[:, c * HL * D:(c + 1) * HL * D].T, dtype=f),
            "kc": np.ascontiguousarray(np.asarray(k_cache)[c, :pos, :], dtype=f),
            "vc": np.ascontiguousarray(np.asarray(v_cache)[c, :pos, :], dtype=f),
            "cosq": cosq, "sinq": sinq, "cosk": cosk, "sink": sink,
        })
    return in_maps


def run(trace=False, **inputs):
    """Build (cached), run on 8 cores, reduce. Returns (out, results)."""
    pos = int(inputs["pos"])
    if pos not in _BUILD_CACHE:
        _BUILD_CACHE[pos] = _build(pos)
    nc = _BUILD_CACHE[pos]
    in_maps = _prep_inputs(**inputs)
    res = run_bass_kernel_spmd(
        nc, in_maps, core_ids=list(range(NC_)), trace=trace)
    part = np.stack([r["out"] for r in res.results])  # (8, T, DIM)
    out = part.sum(axis=0, dtype=np.float32).reshape(1, T, DIM)
    return out, res


def kernel(**inputs):
    out, _ = run(trace=False, **inputs)
    return out
